# revision 6
# baseline (speedup 1.0000x reference)
"""ConvSTFT kernel for Trainium2 (Bass/Tile), data-parallel over batch on 8 cores.

Math: out[b, k, f, i] = sum_n xp[b, 320 f + n] * basis[i*513 + k, n]
where xp = x padded with 512 zeros on both sides, f in [0, 501), n in [0, 1024).

Key layout trick: let Xs[p, j] = xp[p + 64 j] (p in [0,128)).  Since
320 = 64*5 and 128 = 64*2, the contraction chunk c (n = 128 c + p) of
frame f reads Xs[p, 2c + 5f] — i.e. every matmul rhs is a stride-5
column view of Xs.  Xs is built on-chip with PE transposes of
overlapping 128-sample windows (hop 64) staged by a single DMA.
No im2col materialization is ever needed.

Matmuls run in float32r (TF32-like) which streams at 1 col/cycle vs
plain fp32's 1/4 rate; accumulation stays fp32 in PSUM.
"""

import numpy as np
from contextlib import ExitStack

import concourse.bass as bass
import concourse.tile as tile
from concourse import bacc, mybir

# problem constants (hardcoded per harness contract)
B, T = 32, 160000
NCORES = 8
BPC = B // NCORES          # batches per core
HOP, NFFT = 320, 1024
BINS, F = 513, 501         # freq bins, frames
FP = 502                   # frames padded to even (fp32r needs even N)
K2 = 2 * BINS              # 1026 basis rows
PAD = NFFT // 2            # 512
NT = 20                    # transpose tiles per batch
JC = NT * 128              # 2560 Xs columns
L = 8192 * (NT - 1) + 64 * 127 + 128   # 163904 padded xp length
MM_DT = mybir.dt.float32r

_STATE: dict = {}


def _build_nc():
    nc = bacc.Bacc(
        "TRN2", target_bir_lowering=False, debug=False, num_devices=NCORES
    )
    f32 = mybir.dt.float32
    xp = nc.dram_tensor("xp", [BPC, L], f32, kind="ExternalInput").ap()
    bt = nc.dram_tensor("bt", [128, 8 * K2], f32, kind="ExternalInput").ap()
    ident = nc.dram_tensor("ident", [128, 128], f32, kind="ExternalInput").ap()
    out = nc.dram_tensor("out", [BPC, BINS, F, 2], f32, kind="ExternalOutput").ap()

    with tile.TileContext(nc) as tc, ExitStack() as ctx:
        const_pool = ctx.enter_context(tc.tile_pool(name="const", bufs=1))
        stag_pool = ctx.enter_context(tc.tile_pool(name="stag", bufs=2))
        xs_pool = ctx.enter_context(tc.tile_pool(name="xs", bufs=2))
        st_pool = ctx.enter_context(tc.tile_pool(name="st", bufs=3))
        tp_pool = ctx.enter_context(tc.tile_pool(name="tp", bufs=2, space="PSUM"))
        acc_pool = ctx.enter_context(tc.tile_pool(name="acc", bufs=2, space="PSUM"))
        px_pool = ctx.enter_context(tc.tile_pool(name="px", bufs=1, space="PSUM"))

        ident_sb = const_pool.tile([128, 128], f32, tag="ident")
        nc.sync.dma_start(ident_sb[:], ident)
        bt_sb = const_pool.tile([128, 8 * K2], MM_DT, tag="bt")
        nc.sync.dma_start(bt_sb[:], bt.bitcast(MM_DT))

        for b in range(BPC):
            # stage overlapping windows: stag[k, 128 t + r] = xp[b, 8192 t + 64 k + r]
            stag = stag_pool.tile([128, JC], f32, tag="stag")
            src = bass.AP(xp.tensor, b * L, [[64, 128], [8192, NT], [1, 128]])
            dst = stag[:].rearrange("k (t r) -> k t r", r=128)
            nc.sync.dma_start(dst, src)

            # transpose to Xs[p, j] = xp[b, p + 64 j]; stored as fp32r
            # (the PSUM->SBUF copy performs the fp32r rounding)
            xs = xs_pool.tile([128, JC], MM_DT, tag="xs")
            for t in range(NT):
                pt = tp_pool.tile([128, 128], f32, tag="tp")
                nc.tensor.transpose(
                    pt[:], stag[:, t * 128:(t + 1) * 128], ident_sb[:]
                )
                nc.vector.tensor_copy(xs[:, t * 128:(t + 1) * 128], pt[:])

            # main matmuls: 4 (real,imag) chunk pairs of 128 rows
            for r in range(4):
                ps_r = acc_pool.tile([128, FP], f32, tag="accr")
                ps_i = acc_pool.tile([128, FP], f32, tag="acci")
                for c in range(8):
                    rhs = xs[:, 2 * c: 2 * c + 5 * FP: 5]
                    lr = bt_sb[:, c * K2 + 128 * r: c * K2 + 128 * r + 128]
                    li = bt_sb[:, c * K2 + 512 + 128 * r: c * K2 + 512 + 128 * r + 128]
                    nc.tensor.matmul(
                        ps_r[:], lr, rhs,
                        start=(c == 0), stop=(c == 7),
                    )
                    nc.tensor.matmul(
                        ps_i[:], li, rhs,
                        start=(c == 0), stop=(c == 7),
                    )
                st = st_pool.tile([128, 2 * F], f32, tag="st")
                nc.vector.tensor_copy(st[:, 0:2 * F:2], ps_r[:, 0:F])
                nc.vector.tensor_copy(st[:, 1:2 * F:2], ps_i[:, 0:F])
                nc.sync.dma_start(out[b, 128 * r:128 * r + 128, :, :], st[:])

            # last row pair: real bin 512 (col 1024) + imag bin 512 (col 1025)
            ps_x = px_pool.tile([2, FP], f32, tag="px")
            for c in range(8):
                rhs = xs[:, 2 * c: 2 * c + 5 * FP: 5]
                lx = bt_sb[:, c * K2 + 1024: c * K2 + 1026]
                nc.tensor.matmul(
                    ps_x[:], lx, rhs,
                    start=(c == 0), stop=(c == 7),
                )
            st_x = st_pool.tile([2, F], f32, tag="stx")
            nc.vector.tensor_copy(st_x[:], ps_x[:, 0:F])
            nc.sync.dma_start(out[b, 512:513, :, 0:1], st_x[0:1, :])
            nc.sync.dma_start(out[b, 512:513, :, 1:2], st_x[1:2, :])

    nc.compile()
    return nc


def _host_prep_basis(basis: np.ndarray):
    # reorder rows so chunks are [real 0:512 | imag 0:512 | real512, imag512]
    order = np.concatenate(
        [np.arange(0, 512), np.arange(513, 1025), [512], [1025]]
    )
    bt = basis[order].T.astype(np.float32)          # [1024, 1026]
    bt_sb = np.ascontiguousarray(
        bt.reshape(8, 128, K2).transpose(1, 0, 2).reshape(128, 8 * K2)
    )
    return bt_sb


def _get_exec():
    """Build (once) and return a cached executor fn(in_maps) -> full output."""
    if "exec" in _STATE:
        return _STATE["exec"]

    from concourse import bass2jax

    nc = _build_nc()

    def run(in_maps):
        res = bass2jax.run_bass_via_pjrt(nc, in_maps, n_cores=NCORES)
        return np.concatenate([r["out"] for r in res], axis=0)

    _STATE["exec"] = run
    return run


def _prep_inputs(x: np.ndarray, basis: np.ndarray):
    xp_all = np.zeros((B, L), np.float32)
    xp_all[:, PAD:PAD + T] = np.asarray(x, np.float32)
    bt_sb = _host_prep_basis(np.asarray(basis, np.float32))
    ident = np.eye(128, dtype=np.float32)
    in_maps = [
        {
            "xp": xp_all[BPC * c:BPC * (c + 1)],
            "bt": bt_sb,
            "ident": ident,
        }
        for c in range(NCORES)
    ]
    return in_maps


def kernel(x: np.ndarray, basis: np.ndarray) -> np.ndarray:
    run = _get_exec()
    in_maps = _prep_inputs(x, basis)
    return run(in_maps)                            # [32, 513, 501, 2]


# revision 10
# speedup vs baseline: 18846.0066x; 18846.0066x over previous
"""ConvSTFT kernel for Trainium2 (Bass/Tile), data-parallel over batch on 8 cores.

Math: out[b, k, f, i] = sum_n xp[b, 320 f + n] * basis[i*513 + k, n]
where xp = x padded with 512 zeros on both sides, f in [0, 501), n in [0, 1024).

Key layout trick: let Xs[p, j] = xp[p + 64 j] (p in [0,128)).  Since
320 = 64*5 and 128 = 64*2, the contraction chunk c (n = 128 c + p) of
frame f reads Xs[p, 2c + 5f] — i.e. every matmul rhs is a stride-5
column view of Xs.  Xs is built on-chip with PE transposes of
overlapping 128-sample windows (hop 64) staged by a single DMA.
No im2col materialization is ever needed.

Matmuls run in float32r (TF32-like) which streams at 1 col/cycle vs
plain fp32's 1/4 rate; accumulation stays fp32 in PSUM.
"""

import numpy as np
from contextlib import ExitStack

import concourse.bass as bass
import concourse.tile as tile
from concourse import bacc, mybir

# problem constants (hardcoded per harness contract)
B, T = 32, 160000
NCORES = 8
BPC = B // NCORES          # batches per core
HOP, NFFT = 320, 1024
BINS, F = 513, 501         # freq bins, frames
FP = 502                   # frames padded to even (fp32r needs even N)
K2 = 2 * BINS              # 1026 basis rows
PAD = NFFT // 2            # 512
NT = 20                    # transpose tiles per batch
JC = NT * 128              # 2560 Xs columns
L = 8192 * (NT - 1) + 64 * 127 + 128   # 163904 padded xp length
MM_DT = mybir.dt.float32r

_STATE: dict = {}


def _build_nc():
    nc = bacc.Bacc(
        "TRN2", target_bir_lowering=False, debug=False, num_devices=NCORES
    )
    f32 = mybir.dt.float32
    xp = nc.dram_tensor("xp", [BPC, L], f32, kind="ExternalInput").ap()
    bt = nc.dram_tensor("bt", [128, 8 * K2], f32, kind="ExternalInput").ap()
    ident = nc.dram_tensor("ident", [128, 128], f32, kind="ExternalInput").ap()
    out = nc.dram_tensor("out", [BPC, BINS, F, 2], f32, kind="ExternalOutput").ap()

    with tile.TileContext(nc) as tc, ExitStack() as ctx:
        const_pool = ctx.enter_context(tc.tile_pool(name="const", bufs=1))
        stag_pool = ctx.enter_context(tc.tile_pool(name="stag", bufs=2))
        xs_pool = ctx.enter_context(tc.tile_pool(name="xs", bufs=2))
        st_pool = ctx.enter_context(tc.tile_pool(name="st", bufs=3))
        tp_pool = ctx.enter_context(tc.tile_pool(name="tp", bufs=2, space="PSUM"))
        acc_pool = ctx.enter_context(tc.tile_pool(name="acc", bufs=2, space="PSUM"))
        px_pool = ctx.enter_context(tc.tile_pool(name="px", bufs=1, space="PSUM"))

        ident_sb = const_pool.tile([128, 128], f32, tag="ident")
        nc.sync.dma_start(ident_sb[:], ident)
        bt_sb = const_pool.tile([128, 8 * K2], MM_DT, tag="bt")
        nc.sync.dma_start(bt_sb[:], bt.bitcast(MM_DT))

        for b in range(BPC):
            # stage overlapping windows: stag[k, 128 t + r] = xp[b, 8192 t + 64 k + r]
            stag = stag_pool.tile([128, JC], f32, tag="stag")
            src = bass.AP(xp.tensor, b * L, [[64, 128], [8192, NT], [1, 128]])
            dst = stag[:].rearrange("k (t r) -> k t r", r=128)
            nc.sync.dma_start(dst, src)

            # transpose to Xs[p, j] = xp[b, p + 64 j]; stored as fp32r
            # (the PSUM->SBUF copy performs the fp32r rounding)
            xs = xs_pool.tile([128, JC], MM_DT, tag="xs")
            for t in range(NT):
                pt = tp_pool.tile([128, 128], f32, tag="tp")
                nc.tensor.transpose(
                    pt[:], stag[:, t * 128:(t + 1) * 128], ident_sb[:]
                )
                nc.vector.tensor_copy(xs[:, t * 128:(t + 1) * 128], pt[:])

            # main matmuls: 4 (real,imag) chunk pairs of 128 rows
            for r in range(4):
                ps_r = acc_pool.tile([128, FP], f32, tag="accr")
                ps_i = acc_pool.tile([128, FP], f32, tag="acci")
                for c in range(8):
                    rhs = xs[:, 2 * c: 2 * c + 5 * FP: 5]
                    lr = bt_sb[:, c * K2 + 128 * r: c * K2 + 128 * r + 128]
                    li = bt_sb[:, c * K2 + 512 + 128 * r: c * K2 + 512 + 128 * r + 128]
                    nc.tensor.matmul(
                        ps_r[:], lr, rhs,
                        start=(c == 0), stop=(c == 7),
                    )
                    nc.tensor.matmul(
                        ps_i[:], li, rhs,
                        start=(c == 0), stop=(c == 7),
                    )
                st = st_pool.tile([128, 2 * F], f32, tag="st")
                nc.vector.tensor_copy(st[:, 0:2 * F:2], ps_r[:, 0:F])
                nc.vector.tensor_copy(st[:, 1:2 * F:2], ps_i[:, 0:F])
                nc.sync.dma_start(out[b, 128 * r:128 * r + 128, :, :], st[:])

            # last row pair: real bin 512 (col 1024) + imag bin 512 (col 1025)
            ps_x = px_pool.tile([2, FP], f32, tag="px")
            for c in range(8):
                rhs = xs[:, 2 * c: 2 * c + 5 * FP: 5]
                lx = bt_sb[:, c * K2 + 1024: c * K2 + 1026]
                nc.tensor.matmul(
                    ps_x[:], lx, rhs,
                    start=(c == 0), stop=(c == 7),
                )
            st_x = st_pool.tile([2, F], f32, tag="stx")
            nc.vector.tensor_copy(st_x[:], ps_x[:, 0:F])
            nc.sync.dma_start(out[b, 512:513, :, 0:1], st_x[0:1, :])
            nc.sync.dma_start(out[b, 512:513, :, 1:2], st_x[1:2, :])

    nc.compile()
    return nc


def _host_prep_basis(basis: np.ndarray):
    # reorder rows so chunks are [real 0:512 | imag 0:512 | real512, imag512]
    order = np.concatenate(
        [np.arange(0, 512), np.arange(513, 1025), [512], [1025]]
    )
    bt = basis[order].T.astype(np.float32)          # [1024, 1026]
    bt_sb = np.ascontiguousarray(
        bt.reshape(8, 128, K2).transpose(1, 0, 2).reshape(128, 8 * K2)
    )
    return bt_sb


def _get_exec():
    """Build (once) and return a cached executor fn(in_maps) -> full output."""
    if "exec" in _STATE:
        return _STATE["exec"]

    from concourse import bass2jax

    nc = _build_nc()

    def run(in_maps):
        res = bass2jax.run_bass_via_pjrt(nc, in_maps, n_cores=NCORES)
        return np.concatenate([r["out"] for r in res], axis=0)

    _STATE["exec"] = run
    return run


def _prep_inputs(x: np.ndarray, basis: np.ndarray):
    xp_all = np.zeros((B, L), np.float32)
    xp_all[:, PAD:PAD + T] = np.asarray(x, np.float32)
    bt_sb = _host_prep_basis(np.asarray(basis, np.float32))
    ident = np.eye(128, dtype=np.float32)
    in_maps = [
        {
            "xp": xp_all[BPC * c:BPC * (c + 1)],
            "bt": bt_sb,
            "ident": ident,
        }
        for c in range(NCORES)
    ]
    return in_maps


def kernel(x: np.ndarray, basis: np.ndarray) -> np.ndarray:
    run = _get_exec()
    in_maps = _prep_inputs(x, basis)
    return run(in_maps)                            # [32, 513, 501, 2]
